# revision 28
# baseline (speedup 1.0000x reference)
"""Trainium2 Bass kernel for an (unscaled-softmax) attention block.

Problem: x:[4,2048,1024] f32, wq/wk/wv:[1024,1024] f32
    q = x@wq; k = x@wk; v = x@wv
    out = softmax(q @ k^T, axis=-1) @ v        (NO 1/sqrt(d) scaling)

Algebraic restructure: scores = q@k^T = x @ (wq wk^T) @ x^T = (x@M) @ x^T
with M = wq wk^T, so the q- and k-projections collapse into a single
y = x@M projection whose score rhs (raw x^T) comes straight from the
host. M is computed on device (128 rows/core) and AllGather'd across
all 8 cores -- the only collective -- hidden under the v-projection.

Sharding: 8 cores = 4 batches x 2 query-halves. Each core computes y
for its OWN 1024 rows and v for the full 2048 rows of its batch. All
row indexing is in LOCAL order (own half first).

Precision (hw-probed): the unscaled scores are ~N(0, 32768^2) with min
top-2 softmax gap ~2.7, so the score path (M, y, scores) needs ~17-bit
accuracy. It uses a bf16x2 split: a = hi(a) + lo(a), a@b ~= ah@bh +
ah@bl + al@bh -- three full-speed bf16 matmuls (1 cyc/row each) with
fp32 PSUM accumulation (probe err_std 0.14 score units). Probed
alternatives, all rejected: native fp32 lowers to 2 hw passes = 4
cyc/row; fp32r runs 1 cyc/row but keeps only ~12.7 bits (err_std 4.8,
argmax flips); fp16 ~11 bits; and any 2-pass split scheme leaves a
>=2^-12-relative cross term -- 3 passes is the floor at 16-bit piece
widths. The v / attention@v path is plain bf16.

Scheduling (from trace analysis of this and prior versions):
- ~24 warmup matmuls on a zeroed tile run during the initial DMA
  window so the PE's HAM clock gate is warm when real work starts.
- The chip's GPIO power brake (k=13/16 clock) engages run-to-run
  regardless of instruction mix (identical probe phases measured 216
  vs 263 ns/matmul on different runs), so cycles are the currency.
- hi(x^T) ships in slice-major layout [TC, P, DT, CH] and streams on
  the otherwise-idle gpsimd queue as 4 x 1MB DMAs at t=0; the same
  tiles serve as phase B's lhs and the hi score rhs in C/D.
- lo(x^T) streams on sync/scalar behind phase A's weights (first
  needed in phase C).
- The gathered M (bf16 hi/lo) streams from DRAM through 3 rotating
  SBUF tiles on the gpsimd queue behind the AllGather; phase C runs
  kk-outer with all 8 PSUM banks as concurrent accumulation groups,
  so it starts before the whole 4MB M has landed.
- The softmax-weight transpose runs on the DMA xbar, not the PE.
"""

import numpy as np

import concourse.bass as bass
import concourse.bacc as bacc
import concourse.tile as tile
from concourse import mybir

F32 = mybir.dt.float32
BF16 = mybir.dt.bfloat16
P = 128


def build_attention(SQ=1024, T=2048, D=1024, ncores=8):
    """Build the single-core Bass program (uniform across all cores).

    Per-core inputs (all layout/precision prep happens on the host):
      xhi [TC, P, DT, CH] hi bf16 of x^T, slice-major, LOCAL row order
      xlo [DT, P, T]      lo bf16 of x^T (residual), LOCAL row order
      wqt [DT, P, 2, P]   wq^T columns for this core's M row-tile, hi/lo
      wkt [DT, P, 2, D]   full wk^T, hi/lo
      wvb [DT, P, D]      wv rows, bf16
    out: [SQ, D] f32 (own query rows)
    """
    CH = 512             # psum chunk (max moving free dim)
    assert SQ % P == 0 and T % P == 0 and D % P == 0
    DT = D // P          # contraction / d_out tiles
    TT = T // P          # t tiles
    QT = SQ // P         # q row tiles
    TC = T // CH         # score chunks per q-tile / x^T slices
    DC = D // CH         # out-dim chunks
    SC = SQ // CH        # own-row chunks

    nc = bacc.Bacc(
        "TRN2", target_bir_lowering=False, debug=False, num_devices=ncores
    )
    xhi_d = nc.dram_tensor("xhi", [TC, P, DT, CH], BF16, kind="ExternalInput")
    xlo_d = nc.dram_tensor("xlo", [DT, P, T], BF16, kind="ExternalInput")
    wqt_d = nc.dram_tensor("wqt", [P, DT, 2, P], BF16, kind="ExternalInput")
    wkth_d = nc.dram_tensor("wkth", [P, DT, D], BF16, kind="ExternalInput")
    wktl_d = nc.dram_tensor("wktl", [P, DT, D], BF16, kind="ExternalInput")
    wv_d = nc.dram_tensor("wvb", [P, DT, D], BF16, kind="ExternalInput")
    out_d = nc.dram_tensor("out", [SQ, D], F32, kind="ExternalOutput")

    from contextlib import ExitStack

    with tile.TileContext(nc) as tc, ExitStack() as ctx:
        # Persistent SBUF tensors (whole-kernel lifetime).
        arena = ctx.enter_context(tc.tile_pool(name="arena", bufs=1))
        xhi_s = [
            arena.tile([P, DT, CH], BF16, tag=f"xh{s}", name=f"xh{s}")
            for s in range(TC)
        ]
        xlo_u = [arena.tile([P, T], BF16, tag=f"xl{d}", name=f"xl{d}") for d in range(DT)]
        vpair = [arena.tile([P, 2, D], BF16, tag=f"vp{i}", name=f"vp{i}") for i in range(TT // 2)]
        v_sb = [vpair[t // 2][:, t % 2, :] for t in range(TT)]

        def xhi_ap(kk, c0, c1):
            """hi(x^T)[kk*P:(kk+1)*P rows, c0:c1 cols] (c0/c1 in one slice)."""
            sl, o0, o1 = c0 // CH, c0 % CH, (c1 - 1) % CH + 1
            assert (c1 - 1) // CH == sl
            return xhi_s[sl][:, kk, o0:o1]

        def split_psum(ps, hi_dst, lo_dst):
            """hi = bf16(ps); lo = bf16(ps - hi)  (fp32 internally)."""
            nc.vector.tensor_copy(hi_dst, ps)
            nc.vector.tensor_sub(lo_dst, ps, hi_dst)

        # Collective bounce buffers (internal DRAM, dep-tracked). The M
        # gather is split into hi and lo collectives: phase C's hi passes
        # only need the first, so C starts ~24us earlier than a monolithic
        # gather would allow (RDH gather: 17us trigger delay + ~74GB/s).
        p_cc = ctx.enter_context(tc.tile_pool(name="cc", bufs=1, space="DRAM"))
        cc_hi_in = p_cc.tile([P, D], BF16, tag="cmhi", name="cmhi")
        cc_lo_in = p_cc.tile([P, D], BF16, tag="cmli", name="cmli")
        cc_hi_out = p_cc.tile([ncores, P, D], BF16, tag="cmho", name="cmho")
        cc_lo_out = p_cc.tile([ncores, P, D], BF16, tag="cmlo", name="cmlo")
        all_group = [list(range(ncores))]

        # rotating M landing tiles for phase C (12KB/partition; stays
        # open to kernel end so pool close order stays LIFO)
        p_murot = ctx.enter_context(tc.tile_pool(name="mur", bufs=4))
        # phase-B wv pool opens before phase A's pools (LIFO: A's close
        # first, wv closes after phase B)
        es_b = ExitStack()
        p_wv = es_b.enter_context(tc.tile_pool(name="wvp", bufs=1))

        # ---- warmup: keep the PE busy while the first weights stream in,
        # so the HAM clock gate is warm when phase A starts ----
        with (
            tc.tile_pool(name="wrm", bufs=1) as p_wrm,
            tc.tile_pool(name="wps", bufs=1, space="PSUM") as p_wps,
        ):
            wz = p_wrm.tile([P, CH], BF16, tag="wz")
            nc.vector.memset(wz, 0)
            wpss = [
                p_wps.tile([P, CH], F32, tag=f"wps{j}", name=f"wps{j}")
                for j in range(2)
            ]
            NW = 36
            for r in range(NW):
                nc.tensor.matmul(
                    wpss[r % 2], wz[:, :P], wz, start=(r < 2), stop=(r >= NW - 2)
                )

        # ---- phase A: M-rows = wq^T-slice^T @ wk^T (triple bf16), split
        # to bf16 hi/lo, then one 8-wide AllGather. ----
        with (
            tc.tile_pool(name="wqp", bufs=1) as p_wq,
            tc.tile_pool(name="wkp", bufs=1) as p_wk,
            tc.tile_pool(name="mst", bufs=1) as p_mst,
            tc.tile_pool(name="aps", bufs=1, space="PSUM") as p_aps,
        ):
            # The early DMA window is dispatch-latency bound (each unit
            # DMA costs ~2-3us of serialized queue latency), so phase A's
            # prefix ships as FEW BIG partition-major DMAs striped across
            # all three queues, hi weights ahead of lo: A's hi pass starts
            # on 2MB, and A's end gates the AllGather -> phase C chain.
            engs3 = [nc.sync, nc.scalar, nc.gpsimd]
            wqall = p_wq.tile([P, DT, 2, P], BF16, tag="wqall")
            wkhall = p_wk.tile([P, DT, D], BF16, tag="wkhall")
            wklall = p_wk.tile([P, DT, D], BF16, tag="wklall")
            nc.sync.dma_start(out=wqall, in_=wqt_d[:, :, :, :])
            thirds = [(0, 3), (3, 6), (6, DT)]
            for e, (j0, j1) in enumerate(thirds):
                engs3[e].dma_start(
                    out=wkhall[:, j0:j1, :], in_=wkth_d[:, j0:j1, :]
                )
            for e, (j0, j1) in enumerate(thirds):
                engs3[e].dma_start(
                    out=wklall[:, j0:j1, :], in_=wktl_d[:, j0:j1, :]
                )
            # hi(x^T) slices ride the gpsimd queue behind its wk^T share
            # (phase B's gate, also the hi score rhs for C/D)
            for sl in range(TC):
                nc.gpsimd.dma_start(out=xhi_s[sl], in_=xhi_d[sl])
            # wv (phase B's other gate) right behind the weights, 2 DMAs
            wvall = p_wv.tile([P, DT, D], BF16, tag="wvall")
            nc.sync.dma_start(out=wvall[:, : DT // 2, :], in_=wv_d[:, : DT // 2, :])
            nc.scalar.dma_start(out=wvall[:, DT // 2 :, :], in_=wv_d[:, DT // 2 :, :])
            wv_bf = [wvall[:, kk, :] for kk in range(DT)]
            # lo(x^T) streams behind everything else (first use: phase C),
            # one 512KB DMA per d-tile
            for d in range(DT):
                eng = nc.sync if d % 2 == 0 else nc.scalar
                eng.dma_start(out=xlo_u[d], in_=xlo_d[d])

            # both column-halves accumulate together with the ch loop INNER,
            # so consecutive matmuls alternate PSUM banks and pipeline
            pss_a = [
                p_aps.tile([P, CH], F32, tag=f"aps{ch}", name=f"aps{ch}")
                for ch in range(2)
            ]
            for ap, bp in ((0, 0), (1, 0), (0, 1)):
                wk = wkhall if bp == 0 else wklall
                for jj in range(DT):
                    for ch in range(2):
                        nc.tensor.matmul(
                            pss_a[ch],
                            wqall[:, jj, ap, :],
                            wk[:, jj, ch * CH : (ch + 1) * CH],
                            start=(jj == 0 and ap == 0 and bp == 0),
                            stop=(jj == DT - 1 and ap == 0 and bp == 1),
                        )
            mst = p_mst.tile([P, 2, D], BF16, tag="mst")
            for ch in range(2):
                split_psum(
                    pss_a[ch],
                    mst[:, 0, ch * CH : (ch + 1) * CH],
                    mst[:, 1, ch * CH : (ch + 1) * CH],
                )
            nc.gpsimd.dma_start(out=cc_hi_in, in_=mst[:, 0, :])
            nc.gpsimd.dma_start(out=cc_lo_in, in_=mst[:, 1, :])
            nc.gpsimd.collective_compute(
                "AllGather",
                mybir.AluOpType.bypass,
                replica_groups=all_group,
                ins=[cc_hi_in[:]],
                outs=[cc_hi_out[:]],
            )
            nc.gpsimd.collective_compute(
                "AllGather",
                mybir.AluOpType.bypass,
                replica_groups=all_group,
                ins=[cc_lo_in[:]],
                outs=[cc_lo_out[:]],
            )

        # ---- phase B: v = x @ wv for the FULL pair batch (bf16, local
        # order); hides the M AllGather ----
        with tc.tile_pool(name="vps", bufs=2, space="PSUM") as p_vps:
            TPS = TT // TC  # t-tiles per slice
            for sl in range(TC):
                for tl in range(TPS):
                    t = sl * TPS + tl
                    pss = [
                        p_vps.tile([P, CH], F32, tag=f"vps{n}", name=f"vps{n}")
                        for n in range(DC)
                    ]
                    for kk in range(DT):
                        lhs = xhi_s[sl][:, kk, tl * P : (tl + 1) * P]
                        for n in range(DC):
                            nc.tensor.matmul(
                                pss[n],
                                lhs,
                                wv_bf[kk][:, n * CH : (n + 1) * CH],
                                start=(kk == 0),
                                stop=(kk == DT - 1),
                            )
                    for n in range(DC):
                        nc.vector.tensor_copy(
                            v_sb[t][:, n * CH : (n + 1) * CH], pss[n]
                        )
        es_b.close()

        # qu (y^T hi/lo, used C..end) allocates after B's wv frees
        p_qu = ctx.enter_context(tc.tile_pool(name="qup", bufs=1))
        qu = [p_qu.tile([P, 2, SQ], BF16, tag=f"q{m}", name=f"q{m}") for m in range(DT)]

        # ---- phase C: y^T-tiles = M-tiles^T @ x^T-own (triple bf16).
        # The gathered M streams from DRAM through rotating tiles on the
        # gpsimd queue, kk-outer with all 8 PSUM banks as concurrent
        # accumulation groups (m-group of 4 x SC=2), so C starts without
        # waiting for the whole 4MB M landing. ----
        MG = 2               # m-tiles per group (4 groups, double-buffered)
        NGRP = DT // MG
        mu_hi, mu_lo = [], []

        def fetch_mu(i, lo):
            """M columns [mg*MG*P : (mg+1)*MG*P] of k-slice kk (hi or lo).
            hi fetches ride sync, lo rides scalar -- NOT gpsimd, whose
            queue is blocked behind the AG-lo collective instruction."""
            mg, kk = divmod(i, DT)
            src = cc_lo_out if lo else cc_hi_out
            lst = mu_lo if lo else mu_hi
            t = p_murot.tile([P, MG * P], BF16, tag="murl" if lo else "murh")
            (nc.scalar if lo else nc.sync).dma_start(
                out=t, in_=src[kk, :, mg * MG * P : (mg + 1) * MG * P]
            )
            lst.append(t)

        fetch_mu(0, False)
        fetch_mu(1, False)
        fetch_mu(0, True)
        fetch_mu(1, True)
        with tc.tile_pool(name="pps", bufs=2, space="PSUM") as p_pps:
            for mg in range(NGRP):
                pss = [
                    p_pps.tile([P, CH], F32, tag=f"pps{j}", name=f"pps{j}")
                    for j in range(MG * SC)
                ]
                # hi pass: ah@bh + ah@bl terms (needs only the hi gather)
                for kk in range(DT):
                    g = mg * DT + kk
                    mu_t = mu_hi[g]
                    for xp in (0, 1):
                        for ml in range(MG):
                            for c in range(SC):
                                rhs = (
                                    xhi_ap(kk, c * CH, (c + 1) * CH)
                                    if xp == 0
                                    else xlo_u[kk][:, c * CH : (c + 1) * CH]
                                )
                                nc.tensor.matmul(
                                    pss[ml * SC + c],
                                    mu_t[:, ml * P : (ml + 1) * P],
                                    rhs,
                                    start=(kk == 0 and xp == 0),
                                    stop=False,
                                )
                    if g + 2 < NGRP * DT:
                        fetch_mu(g + 2, False)
                # lo pass: al@bh terms (needs the lo gather)
                for kk in range(DT):
                    g = mg * DT + kk
                    mu_t = mu_lo[g]
                    for ml in range(MG):
                        for c in range(SC):
                            nc.tensor.matmul(
                                pss[ml * SC + c],
                                mu_t[:, ml * P : (ml + 1) * P],
                                xhi_ap(kk, c * CH, (c + 1) * CH),
                                start=False,
                                stop=(kk == DT - 1),
                            )
                    if g + 2 < NGRP * DT:
                        fetch_mu(g + 2, True)
                for ml in range(MG):
                    m = mg * MG + ml
                    for c in range(SC):
                        split_psum(
                            pss[ml * SC + c],
                            qu[m][:, 0, c * CH : (c + 1) * CH],
                            qu[m][:, 1, c * CH : (c + 1) * CH],
                        )

        # ---- phase D: per q-tile attention, one-stage software pipeline:
        # PE runs scores(qi), then AV of qi-1 while ACT exponentiates qi and
        # the DMA xbar transposes qi's softmax weights. ----
        with (
            tc.tile_pool(name="stats", bufs=4) as p_st,
            tc.tile_pool(name="ssb", bufs=2) as p_ssb,
            tc.tile_pool(name="exps", bufs=2) as p_ex,
            tc.tile_pool(name="wtsb", bufs=2) as p_wtsb,
            tc.tile_pool(name="osb", bufs=2) as p_o,
            tc.tile_pool(name="scps", bufs=1, space="PSUM") as p_sc,
            tc.tile_pool(name="avps", bufs=1, space="PSUM") as p_av,
        ):

            def emit_scores(qi):
                ssb = p_ssb.tile([P, T], F32, tag="ssb")
                for c in range(TC):
                    scs[c] = p_sc.tile([P, CH], F32, tag=f"sc{c}", name=f"sc{c}")
                for kk in range(DT):
                    for qp, kp in ((0, 0), (0, 1), (1, 0)):
                        lhs = qu[kk][:, qp, qi * P : (qi + 1) * P]
                        for c in range(TC):
                            rhs = (
                                xhi_ap(kk, c * CH, (c + 1) * CH)
                                if kp == 0
                                else xlo_u[kk][:, c * CH : (c + 1) * CH]
                            )
                            nc.tensor.matmul(
                                scs[c],
                                lhs,
                                rhs,
                                start=(kk == 0 and qp == 0 and kp == 0),
                                stop=(kk == DT - 1 and qp == 1),
                            )
                for c in range(TC):
                    nc.vector.tensor_copy(
                        ssb[:, c * CH : (c + 1) * CH], scs[c]
                    )
                return ssb

            def emit_softmax(qi, ssb):
                mx4 = p_st.tile([P, TC], F32, tag="mx4")
                for c in range(TC):
                    nc.vector.reduce_max(
                        mx4[:, c : c + 1],
                        ssb[:, c * CH : (c + 1) * CH],
                        axis=mybir.AxisListType.X,
                    )
                negmx = p_st.tile([P, 1], F32, tag="negmx")
                mx = p_st.tile([P, 1], F32, tag="mx")
                nc.vector.reduce_max(mx, mx4, axis=mybir.AxisListType.X)
                nc.scalar.mul(negmx, mx, -1.0)
                sums = p_st.tile([P, TC], F32, tag="sums")
                exps = p_ex.tile([P, T], BF16, tag="exps")
                wt = p_wtsb.tile([P, TT, P], BF16, tag="wt")
                TPS = TT // TC
                tr_engs = [nc.sync, nc.scalar, nc.sync, nc.scalar]
                for c in range(TC):
                    nc.scalar.activation(
                        out=exps[:, c * CH : (c + 1) * CH],
                        in_=ssb[:, c * CH : (c + 1) * CH],
                        func=mybir.ActivationFunctionType.Exp,
                        bias=negmx[:, 0:1],
                        scale=1.0,
                        accum_out=sums[:, c : c + 1],
                    )
                    # [s, t]->[t, s] transpose on the DMA xbar, chunked per
                    # exp chunk and spread across the dispatch queues so the
                    # next AV never waits on one serial 0.5MB transpose:
                    # wt[p, c*TPS+k, j] = exps[j, c*CH + k*P + p]
                    tr_engs[c].dma_start_transpose(
                        wt[:, c * TPS : (c + 1) * TPS, :],
                        exps[:, c * CH : (c + 1) * CH],
                    )
                ssum = p_st.tile([P, 1], F32, tag="ssum")
                nc.vector.reduce_sum(ssum, sums, axis=mybir.AxisListType.X)
                rsum = p_st.tile([P, 1], F32, tag="rsum")
                nc.vector.reciprocal(rsum, ssum)
                return wt, rsum

            def emit_av(qi, wt, rsum, last=False):
                avs = [
                    p_av.tile([P, CH], F32, tag=f"av{n}", name=f"av{n}")
                    for n in range(DC)
                ]
                osb = p_o.tile([P, D], F32, tag="o")
                if not last:
                    for t in range(TT):
                        lhs = wt[:, t, :]
                        for n in range(DC):
                            nc.tensor.matmul(
                                avs[n],
                                lhs,
                                v_sb[t][:, n * CH : (n + 1) * CH],
                                start=(t == 0),
                                stop=(t == TT - 1),
                            )
                    for n in range(DC):
                        nc.vector.tensor_scalar_mul(
                            osb[:, n * CH : (n + 1) * CH], avs[n], rsum[:, 0:1]
                        )
                    nc.scalar.dma_start(
                        out=out_d[qi * P : (qi + 1) * P, :], in_=osb
                    )
                    return
                # last q-tile: run the two output chunks as sequential
                # chains so chunk 0's scale+store overlaps chunk 1's
                # matmuls (shaves the pipeline tail)
                for n in range(DC):
                    for t in range(TT):
                        nc.tensor.matmul(
                            avs[n],
                            wt[:, t, :],
                            v_sb[t][:, n * CH : (n + 1) * CH],
                            start=(t == 0),
                            stop=(t == TT - 1),
                        )
                    nc.vector.tensor_scalar_mul(
                        osb[:, n * CH : (n + 1) * CH], avs[n], rsum[:, 0:1]
                    )
                    nc.scalar.dma_start(
                        out=out_d[qi * P : (qi + 1) * P, n * CH : (n + 1) * CH],
                        in_=osb[:, n * CH : (n + 1) * CH],
                    )

            scs = [None] * TC
            prev = None
            for qi in range(QT):
                ssb = emit_scores(qi)
                if prev is not None:
                    emit_av(*prev)
                wt, rsum = emit_softmax(qi, ssb)
                prev = (qi, wt, rsum)
            emit_av(*prev, last=True)

    nc.compile()
    return nc


_CACHE = {}


def _built_full():
    if "nc" not in _CACHE:
        _CACHE["nc"] = build_attention(1024, 2048, 1024)
    return _CACHE["nc"]


def _bf16_split(a):
    """fp32 array -> (hi, lo) bf16 with hi + lo ~= a (RNE, matches DVE)."""
    import ml_dtypes

    hi = a.astype(ml_dtypes.bfloat16)
    lo = (a - hi.astype(np.float32)).astype(ml_dtypes.bfloat16)
    return hi, lo


def host_prep_x(x_rows, P=128, CH=512):
    """x rows [XR, D] f32 -> hi bf16 x^T slice-major [XR//CH, P, DT, CH]
    and lo bf16 x^T [DT, P, XR]."""
    XR, D = x_rows.shape
    xT = np.ascontiguousarray(x_rows.T.astype(np.float32))  # [D, XR]
    hi, lo = _bf16_split(xT)
    # hi: [D, XR] -> [DT, P, TC, CH] -> [TC, P, DT, CH]
    xhi = np.ascontiguousarray(
        hi.reshape(D // P, P, XR // CH, CH).transpose(2, 1, 0, 3)
    )
    xlo = np.ascontiguousarray(lo.reshape(D // P, P, XR))
    return xhi, xlo


def host_prep_wT(w, c0=None, c1=None, P=128):
    """w [D, D] f32 -> w^T cols [c0:c1] as [DT, P, 2, c1-c0] bf16 hi/lo."""
    D = w.shape[0]
    wT = np.ascontiguousarray(w.astype(np.float32).T)  # [j, a]
    if c0 is not None:
        wT = wT[:, c0:c1]
    hi, lo = _bf16_split(wT)
    out = np.stack([hi, lo], axis=1).reshape(D // P, P, 2, wT.shape[1])
    return np.ascontiguousarray(out)


def host_prep_wv(wv, P=128):
    import ml_dtypes

    D = wv.shape[0]
    return np.ascontiguousarray(
        wv.astype(np.float32).astype(ml_dtypes.bfloat16).reshape(D // P, P, D)
    )


def _make_in_maps(x, wq, wk, wv):
    """Per-core input maps: core c = (batch c//2, query-half c%2)."""
    x = np.ascontiguousarray(np.asarray(x, dtype=np.float32))
    wq = np.asarray(wq, dtype=np.float32)
    wk = np.asarray(wk, dtype=np.float32)
    wv = np.asarray(wv, dtype=np.float32)
    B, S, D = x.shape
    half = S // 2
    wkt = host_prep_wT(wk)  # [DT, P, 2, D]
    wkth = np.ascontiguousarray(wkt[:, :, 0, :].transpose(1, 0, 2))
    wktl = np.ascontiguousarray(wkt[:, :, 1, :].transpose(1, 0, 2))
    wvb = np.ascontiguousarray(host_prep_wv(wv).transpose(1, 0, 2))
    in_maps = []
    for c in range(8):
        b, h = divmod(c, 2)
        if h == 0:
            xloc = x[b]
        else:
            xloc = np.concatenate([x[b][half:], x[b][:half]], axis=0)
        xhi, xlo = host_prep_x(xloc)
        in_maps.append(
            {
                "xhi": xhi,
                "xlo": xlo,
                "wqt": np.ascontiguousarray(
                    host_prep_wT(wq, c * P, (c + 1) * P).transpose(1, 0, 2, 3)
                ),
                "wkth": wkth,
                "wktl": wktl,
                "wvb": wvb,
            }
        )
    return in_maps, (B, S, D)


def _assemble(results, shape):
    B, S, D = shape
    half = S // 2
    out = np.empty((B, S, D), np.float32)
    for c in range(8):
        b, h = divmod(c, 2)
        out[b, h * half : (h + 1) * half] = results[c]["out"]
    return out


def kernel(x, wq, wk, wv):
    """Full (unsharded) inputs -> full output, running SPMD on 8 cores."""
    from concourse.bass_utils import run_bass_kernel_spmd

    in_maps, shape = _make_in_maps(x, wq, wk, wv)
    nc = _built_full()
    res = run_bass_kernel_spmd(nc, in_maps, core_ids=list(range(8))).results
    return _assemble(res, shape)
